# revision 1
# baseline (speedup 1.0000x reference)
"""CompressionAttention TRN2 Bass kernel (8 NeuronCores, SPMD).

Sharding: core c handles batch b = c//4 and heads [4*(c%4), 4*(c%4)+4).
Each core computes its 4 heads' attention output and a partial output
projection (S, D); the host sums the 4 partials per batch and adds bo.

Math (validated against the jax reference in fp32, rel err ~6e-7):
  Projections are folded on the host:
    qT   = (scale*Wq_sl) x^T          (transposed layout, dlocal x S)
    kT   = Wk_sl x^T
    vkT  = (Wk_sl @ Wv) x^T           (v_d_k folded:  (x Wv^T) Wk^T = x (Wk Wv)^T)
    vk,vv natural = x (Wfold_sl)^T    (token-major)
  Per head:  w = exp(q_d_h @ k^T)  (C,S)  -- max subtraction provably cancels
             norm = cumsum_s w;  up = U / norm^T;  r = softmax_c(up) / norm^T
             U[s,c] = sum_{t<=s} (q_s . vk_t) w[c,t]  via running SBUF state
               KVW[d,c] = sum_{t < blk} vk[t,d] w[c,t]  plus masked diag block
             out^T[d,s] = sum_c KV2[c,d] r[s,c] + sum_{t in blk, t<=s} vv[t,d] MT[t,s]
               KV2[c,d] = running sum w[c,t] vv[t,c->d];  MT[t,s] = sum_c w[c,t] r[s,c]
  partial = out_local @ Wo[:, hsl]  summed across the 4 cores of the batch group.

Matmuls run in float32r (TF32-like, ~1.6e-4/matmul, full PE rate at moving
dim >= 256); PSUM accumulates fp32. Hardware constraints honored here:
  - fp32r matmul outputs must start at partition 0 (head-packed free-dim
    layouts; partition shifts done with SBUF->SBUF DMAs).
  - PSUM has_written clears (start=True) are bank-wide: every bank hosts
    either a single multi-matmul accumulation group at a time, or only
    single-shot (start&stop) matmuls. Cross-block running state lives in
    SBUF (fp32) and is updated with per-block single-shot PSUM deltas.
"""
import math
import os
import sys

import numpy as np

for _p in ("/opt/trn_rl_repo",):
    if _p not in sys.path and os.path.isdir(_p):
        sys.path.append(_p)

import concourse.bass as bass
import concourse.tile as tile
from concourse import bacc, mybir
from concourse.bass_utils import run_bass_kernel_spmd
from concourse.masks import make_identity, make_upper_triangular

F32 = mybir.dt.float32
F32R = mybir.dt.float32r
EXP = mybir.ActivationFunctionType.Exp

S, B, D = 1024, 2, 1024
H, DH, C = 16, 64, 32
HPC = 4            # heads per core
DL = HPC * DH      # local head dims = 256
BS = 128           # s-block size
NB = S // BS       # 8 blocks
KT = D // 128      # 8 k-tiles over D
SCALE = 1.0 / math.sqrt(DH)
SLOT = (0, 2, 1, 3)
BF16 = os.environ.get("KBF16", "1") == "1"
BF = mybir.dt.bfloat16   # head p -> slot index in qT2/kT2/vkT2 (shuffle-DMA layout)


def _build_nc():
    nc = bacc.Bacc()

    DT_IN = BF if BF16 else F32R
    xt_d = nc.declare_dram_parameter("xT", [D, S], DT_IN, isOutput=False)
    wq_d = nc.declare_dram_parameter("wqT", [D, DL], DT_IN, isOutput=False)
    wk_d = nc.declare_dram_parameter("wkT", [D, DL], DT_IN, isOutput=False)
    wvk_d = nc.declare_dram_parameter("wvkT", [D, DL], DT_IN, isOutput=False)
    wvv_d = nc.declare_dram_parameter("wvvT", [D, DL], DT_IN, isOutput=False)
    wo_d = nc.declare_dram_parameter("woT", [64, HPC, D], F32R, isOutput=False)
    qd_d = nc.declare_dram_parameter("qd", [64, HPC, C], F32R, isOutput=False)
    biasT_d = nc.declare_dram_parameter("biasT", [128, 6], F32, isOutput=False)
    biasN_d = nc.declare_dram_parameter("biasN", [1, 2, DL], F32R, isOutput=False)
    out_d = nc.declare_dram_parameter("partial", [S, D], F32, isOutput=True)

    with tile.TileContext(nc) as tc:
        with tc.tile_pool(name="persist", bufs=1) as pp, \
             tc.tile_pool(name="sloop", bufs=2) as sl:
            # ---- constants ----
            ident_f = pp.tile([128, 128], F32)
            make_identity(nc, ident_f)
            triu = pp.tile([128, 128], F32)
            make_upper_triangular(nc, triu, val=1.0, diag=True)
            ones_f = pp.tile([1, 128], F32)
            nc.vector.memset(ones_f, 1.0)
            ones_col = pp.tile([1, 128], F32R)
            nc.vector.tensor_copy(ones_col, ones_f)

            # ---- persistent SBUF (live through attention) ----
            DT_Q = BF if BF16 else F32R
            qT2 = pp.tile([64, HPC, S], DT_Q)
            vkT2 = pp.tile([64, HPC, S], DT_Q)
            vkn = pp.tile([128, NB, DL], F32R)
            vvn = pp.tile([128, NB, DL], F32R)
            w_head = pp.tile([32, HPC, S], F32R)    # w native, per-head at base 0
            if BF16:
                w_head_m = pp.tile([32, HPC, S], BF, name="w_head_m")
                vvn_m = pp.tile([128, NB, DL], BF, name="vvn_m")
            else:
                w_head_m = w_head
                vvn_m = None
            wT = pp.tile([128, NB, 128], F32R)      # (s, (head, c)) tiles
            rn = pp.tile([128, NB, 128], F32)       # 1/norm^T, same layout
            outT = pp.tile([64, HPC, NB, 128], F32R)

            # ================= Phases 1+2: projections, w, norm =================
            with tc.tile_pool(name="p12", bufs=1) as p12, \
                 tc.tile_pool(name="wpool", bufs=2) as wp, \
                 tc.tile_pool(name="stg", bufs=2) as stg:
                xt = p12.tile([128, KT, S], BF if BF16 else F32R)
                xt_src = xt_d.rearrange("(kt p) s -> kt p s", p=128)
                for kt in range(KT):
                    eng = nc.gpsimd if kt in (1, 4, 7) else nc.sync
                    eng.dma_start(out=xt[:, kt, :], in_=xt_src[kt])
                qd = p12.tile([64, HPC, C], F32R)
                nc.sync.dma_start(out=qd, in_=qd_d[:, :, :])
                biasT = p12.tile([128, 6], F32)
                nc.sync.dma_start(out=biasT, in_=biasT_d[:, :])
                biasN = p12.tile([1, 2, DL], F32R)
                nc.sync.dma_start(out=biasN, in_=biasN_d[:, :, :])
                kT2 = p12.tile([64, HPC, S], F32R)
                norm_head = p12.tile([32, HPC, S], F32)

                with tc.tile_pool(name="ps1", bufs=2, space="PSUM") as ps1, \
                     tc.tile_pool(name="ps1b", bufs=1, space="PSUM") as ps1b:
                    wvk_sb = None
                    for ti, (wdram, dst2) in enumerate(
                            ((wq_d, qT2), (wk_d, kT2), (wvk_d, vkT2))):
                        wsb = wp.tile([128, KT, DL], BF if BF16 else F32R, tag="wtile")
                        w_src = wdram.rearrange("(kt p) n -> kt p n", p=128)
                        for half in range(2):
                            nc.scalar.dma_start(
                                out=wsb[:, 4 * half:4 * half + 4, :],
                                in_=w_src[4 * half:4 * half + 4].rearrange(
                                    "kt p n -> p kt n"))
                        if ti == 2:
                            wvk_sb = wsb
                        for ch in range(2):
                            stg_t = stg.tile([128, 2, 512], F32R, tag="stage")
                            for pt in range(2):
                                ps = ps1.tile([128, 512], F32, tag="pst")
                                for kt in range(KT):
                                    nc.tensor.matmul(
                                        ps,
                                        lhsT=wsb[:, kt, 128 * pt:128 * pt + 128],
                                        rhs=xt[:, kt, 512 * ch:512 * ch + 512],
                                        start=(kt == 0), stop=(kt == KT - 1))
                                nc.vector.tensor_scalar_add(
                                    out=stg_t[:, pt, :],
                                    in0=ps,
                                    scalar1=biasT[:, 2 * ti + pt:2 * ti + pt + 1])
                            if BF16 and ti != 1:
                                stg_bf = stg.tile([128, 2, 512], BF, tag="stagebf")
                                nc.vector.tensor_copy(stg_bf, stg_t)
                                src_t = stg_bf
                            else:
                                src_t = stg_t
                            # head-aligned placement via SBUF->SBUF DMA
                            # slots (0,1) <- rows 0:64 of (pt0, pt1); (2,3) <- rows 64:128
                            nc.gpsimd.dma_start(
                                out=dst2[:, 0:2, 512 * ch:512 * ch + 512],
                                in_=src_t[0:64, :, :])
                            nc.sync.dma_start(
                                out=dst2[:, 2:4, 512 * ch:512 * ch + 512],
                                in_=src_t[64:128, :, :])
                        if ti == 1:
                            # w, norm per head as soon as kT2 is ready --
                            # overlaps the vkT projection matmuls
                            for p in range(HPC):
                                dps = ps1b.tile([32, S], F32, tag="down")
                                for ch2 in range(2):
                                    nc.tensor.matmul(
                                        dps[:, 512 * ch2:512 * ch2 + 512],
                                        lhsT=qd[:, p, :],
                                        rhs=kT2[:, SLOT[p], 512 * ch2:512 * ch2 + 512],
                                        start=True, stop=True)
                                nc.scalar.activation(w_head[:, p, :], dps, func=EXP)
                                if BF16:
                                    nc.scalar.activation(w_head_m[:, p, :], dps, func=EXP)
                                nc.vector.tensor_tensor_scan(
                                    out=norm_head[:, p, :], data0=w_head[:, p, :],
                                    data1=w_head[:, p, :], initial=0.0,
                                    op0=mybir.AluOpType.add, op1=mybir.AluOpType.bypass)

                    for i in range(NB):
                        dtp = ps1b.tile([128, 128], F32, tag="dT")
                        for p in range(HPC):
                            nc.tensor.matmul(
                                dtp[:, 32 * p:32 * p + 32],
                                lhsT=kT2[:, SLOT[p], 128 * i:128 * i + 128],
                                rhs=qd[:, p, :],
                                start=True, stop=True)
                        nc.scalar.activation(wT[:, i, :], dtp, func=EXP)
                        # norm^T via per-head PE transpose + reciprocal
                        ntp = ps1b.tile([128, 32 * HPC], F32, tag="nT")
                        for p in range(HPC):
                            nc.tensor.transpose(
                                ntp[:, 32 * p:32 * p + 32],
                                norm_head[:, p, 128 * i:128 * i + 128],
                                ident_f[0:32, 0:32])
                        nc.vector.reciprocal_approx_fast(out=rn[:, i, :], in_=ntp)
                    wvv_sb = wp.tile([128, KT, DL], BF if BF16 else F32R, tag="wtile")
                    wvv_src = wvv_d.rearrange("(kt p) n -> kt p n", p=128)
                    for half in range(2):
                        nc.scalar.dma_start(
                            out=wvv_sb[:, 4 * half:4 * half + 4, :],
                            in_=wvv_src[4 * half:4 * half + 4].rearrange(
                                "kt p n -> p kt n"))
                    # natural-layout vk, vv (token-major); one group per bank
                    for tt in range(NB):
                        psk = ps1b.tile([128, DL], F32, tag="psnk")
                        psv = ps1b.tile([128, DL], F32, tag="psnv")
                        for kt in range(KT):
                            nc.tensor.matmul(
                                psk,
                                lhsT=xt[:, kt, 128 * tt:128 * tt + 128],
                                rhs=wvk_sb[:, kt, :],
                                start=(kt == 0), stop=False)
                            nc.tensor.matmul(
                                psv,
                                lhsT=xt[:, kt, 128 * tt:128 * tt + 128],
                                rhs=wvv_sb[:, kt, :],
                                start=(kt == 0), stop=False)
                        nc.tensor.matmul(psk, lhsT=ones_col,
                                         rhs=biasN[0:1, 0, :], start=False, stop=True)
                        nc.tensor.matmul(psv, lhsT=ones_col,
                                         rhs=biasN[0:1, 1, :], start=False, stop=True)
                        nc.vector.tensor_copy(vkn[:, tt, :], psk)
                        nc.vector.tensor_copy(vvn[:, tt, :], psv)
                        if BF16:
                            nc.vector.tensor_copy(vvn_m[:, tt, :], psv)

            # ================= Phase 3: blocked attention =================
            # PSUM banks: delta(1) u(1) u1(1) ot1(1) ot2(1) rt(1) + gt(1) mt(1) = 8
            with tc.tile_pool(name="ps2", bufs=1, space="PSUM") as ps2, \
                 tc.tile_pool(name="ps2b", bufs=1, space="PSUM") as ps2b, \
                 tc.tile_pool(name="ps2c", bufs=2, space="PSUM") as ps2c, \
                 tc.tile_pool(name="wopool", bufs=1) as wop, \
                 tc.tile_pool(name="popool", bufs=2) as pop:
                wo2 = wop.tile([64, HPC, D], F32R)
                nc.gpsimd.dma_start(out=wo2[:, 0:2, :], in_=wo_d[:, 0:2, :])
                nc.sync.dma_start(out=wo2[:, 2:4, :], in_=wo_d[:, 2:4, :])
                kvw_f = None   # running states, SBUF fp32 + fp32r casts
                kv2_f = None
                kvw_r = None
                kv2_r = None
                for i in range(NB):
                    # per-block state deltas: single-shot matmuls, one shared bank
                    dl_ps = ps2.tile([64, 384], F32, tag="delta")
                    for p in range(HPC):
                        nc.tensor.matmul(
                            dl_ps[0:64, 32 * p:32 * p + 32],
                            lhsT=vkn[:, i, 64 * p:64 * p + 64],
                            rhs=wT[:, i, 32 * p:32 * p + 32],
                            start=True, stop=True)
                        nc.tensor.matmul(
                            dl_ps[0:32, 128 + 64 * p:128 + 64 * p + 64],
                            lhsT=wT[:, i, 32 * p:32 * p + 32],
                            rhs=vvn[:, i, 64 * p:64 * p + 64],
                            start=True, stop=True)
                    # U = U1 (start) + Udiag (stop): one group per head region,
                    # sequential same-bank groups (bank-wide clears are safe
                    # once the previous group has stopped; same-bank matmuls
                    # execute in trace order)
                    u_ps = ps2.tile([128, 128], F32, tag="u")
                    for p in range(HPC):
                        gt_ps = ps2c.tile([128, 128], F32, tag="gt")
                        nc.tensor.matmul(
                            gt_ps,
                            lhsT=vkT2[:, SLOT[p], 128 * i:128 * i + 128],
                            rhs=qT2[:, SLOT[p], 128 * i:128 * i + 128],
                            start=True, stop=True)
                        gt_sb = sl.tile([128, 128], F32R, tag="gtsb")
                        nc.vector.tensor_mul(gt_sb, gt_ps, triu)
                        if i > 0:
                            nc.tensor.matmul(
                                u_ps[:, 32 * p:32 * p + 32],
                                lhsT=qT2[:, SLOT[p], 128 * i:128 * i + 128],
                                rhs=kvw_r[0:64, 32 * p:32 * p + 32],
                                start=True, stop=False)
                        nc.tensor.matmul(
                            u_ps[:, 32 * p:32 * p + 32],
                            lhsT=gt_sb,
                            rhs=wT[:, i, 32 * p:32 * p + 32],
                            start=(i == 0), stop=True)
                    # softmax over c (32 per head), folded with 1/norm factors
                    up_sb = sl.tile([128, 128], F32, tag="up")
                    nc.vector.tensor_mul(up_sb, u_ps, rn[:, i, :])
                    e_sb = sl.tile([128, 128], F32, tag="e")
                    nc.scalar.activation(e_sb, up_sb, func=EXP)
                    sum_sb = sl.tile([128, HPC], F32, tag="sum")
                    nc.vector.tensor_reduce(
                        sum_sb, e_sb[:, :].rearrange("a (h c) -> a h c", h=HPC),
                        axis=mybir.AxisListType.X, op=mybir.AluOpType.add)
                    inv_sb = sl.tile([128, HPC], F32, tag="inv")
                    nc.vector.reciprocal(inv_sb, sum_sb)
                    r_sb = sl.tile([128, 128], F32, tag="r")
                    for p in range(HPC):
                        nc.vector.scalar_tensor_tensor(
                            out=r_sb[:, 32 * p:32 * p + 32],
                            in0=e_sb[:, 32 * p:32 * p + 32],
                            scalar=inv_sb[:, p:p + 1],
                            in1=rn[:, i, 32 * p:32 * p + 32],
                            op0=mybir.AluOpType.mult,
                            op1=mybir.AluOpType.mult)
                    # r^T per head (base-0): PE transpose of (128, 32) slices
                    rt_ps = ps2.tile([32, HPC, 128], F32, tag="rt")
                    for p in range(HPC):
                        nc.tensor.transpose(
                            rt_ps[:, p, :], r_sb[:, 32 * p:32 * p + 32], ident_f)
                    rt_sb = sl.tile([32, HPC, 128], BF if BF16 else F32R, tag="rtsb")
                    nc.scalar.copy(rt_sb, rt_ps)
                    # out^T: term1 (prefix KV2, start) + term2 (diag, stop)
                    ot_ps = ps2.tile([64, HPC, 128], F32, tag="ot")
                    for p in range(HPC):
                        mt_ps = ps2b.tile([128, 128], F32, tag="mt")
                        nc.tensor.matmul(
                            mt_ps,
                            lhsT=w_head_m[:, p, 128 * i:128 * i + 128],
                            rhs=rt_sb[:, p, :],
                            start=True, stop=True)
                        mt_sb = sl.tile([128, 128], BF if BF16 else F32R, tag="mtsb")
                        nc.vector.tensor_mul(mt_sb, mt_ps, triu)
                        if i > 0:
                            nc.tensor.matmul(
                                ot_ps[:, p, :],
                                lhsT=kv2_r[0:32, 64 * p:64 * p + 64],
                                rhs=rt_sb[:, p, :],
                                start=True, stop=False)
                        nc.tensor.matmul(
                            ot_ps[:, p, :],
                            lhsT=(vvn_m if BF16 else vvn)[:, i, 64 * p:64 * p + 64],
                            rhs=mt_sb,
                            start=(i == 0), stop=True)
                    if i >= 5:
                        nc.vector.tensor_copy(outT[:, :, i, :], ot_ps)
                    else:
                        nc.scalar.copy(outT[:, :, i, :], ot_ps)
                    # interleaved output projection for token tile tt == i
                    for ch in range(2):
                        po = ps2.tile([128, 512], F32, tag="po")
                        for hh in range(HPC):
                            nc.tensor.matmul(
                                po,
                                lhsT=outT[:, hh, i, :],
                                rhs=wo2[:, hh, 512 * ch:512 * ch + 512],
                                start=(hh == 0), stop=(hh == HPC - 1))
                        po_sb = pop.tile([128, 512], F32, tag="posb")
                        nc.scalar.copy(po_sb, po)
                        nc.sync.dma_start(
                            out=out_d[128 * i:128 * i + 128,
                                      512 * ch:512 * ch + 512],
                            in_=po_sb)
                    # advance running states (SBUF fp32) and cast for matmul use
                    if i < NB - 1:
                        kvw_new = sl.tile([64, 128], F32, tag="kvwf")
                        kv2_new = sl.tile([32, 256], F32, tag="kv2f")
                        if i == 0:
                            nc.vector.tensor_copy(kvw_new, dl_ps[0:64, 0:128])
                            nc.vector.tensor_copy(kv2_new, dl_ps[0:32, 128:384])
                        else:
                            nc.vector.tensor_add(kvw_new, kvw_f, dl_ps[0:64, 0:128])
                            nc.vector.tensor_add(kv2_new, kv2_f, dl_ps[0:32, 128:384])
                        kvw_f, kv2_f = kvw_new, kv2_new
                        kvw_r = sl.tile([64, 128], BF if BF16 else F32R, tag="kvwr")
                        nc.scalar.copy(kvw_r, kvw_f)
                        kv2_r = sl.tile([32, 256], BF if BF16 else F32R, tag="kv2r")
                        nc.scalar.copy(kv2_r, kv2_f)


    nc.finalize()
    return nc


_NC_CACHE = {}
_last_in_maps = None


def _get_nc():
    if "nc" not in _NC_CACHE:
        _NC_CACHE["nc"] = _build_nc()
    return _NC_CACHE["nc"]


def kernel(x, q_down, Wq, bq, Wk, bk, Wv, bv, Wo, bo):
    x = np.asarray(x, np.float32)
    q_down = np.asarray(q_down, np.float32)
    Wq = np.asarray(Wq, np.float32); bq = np.asarray(bq, np.float32)
    Wk = np.asarray(Wk, np.float32); bk = np.asarray(bk, np.float32)
    Wv = np.asarray(Wv, np.float32); bv = np.asarray(bv, np.float32)
    Wo = np.asarray(Wo, np.float32); bo = np.asarray(bo, np.float32)

    nc = _get_nc()
    qd_r = q_down.reshape(C, H, DH)

    in_maps = []
    for core in range(8):
        b = core // 4
        h0 = HPC * (core % 4)
        hsl = slice(h0 * DH, (h0 + HPC) * DH)

        xT = np.ascontiguousarray(x[:, b, :].T)                     # (D, S)
        wqT = np.ascontiguousarray((SCALE * Wq[hsl, :]).T)          # (D, DL)
        wkT = np.ascontiguousarray(Wk[hsl, :].T)
        wvkT = np.ascontiguousarray((Wk[hsl, :] @ Wv).T)
        wvvT = np.ascontiguousarray((Wv[hsl, :] @ Wv).T)
        # woT head-packed: (64, HPC, D); head h slab = Wo[:, that head's dims].T
        woT = np.ascontiguousarray(
            Wo[:, hsl].T.reshape(HPC, 64, D).transpose(1, 0, 2))
        # qd head-packed: (64, HPC, C), all heads at base partition 0
        qd_pack = np.ascontiguousarray(
            np.stack([SCALE * qd_r[:, h0 + p, :].T for p in range(HPC)], axis=1))

        # biases: proj-T per-partition bias columns (q, k, vk folded)
        bq_sl = SCALE * bq[hsl]
        bk_sl = bk[hsl]
        bvk_sl = Wk[hsl, :] @ bv + bk_sl          # bias of folded v_d_k
        bvv_sl = Wv[hsl, :] @ bv + bv[hsl]        # bias of folded v_d_v
        biasT = np.zeros((128, 6), np.float32)
        for ti, bvec in enumerate((bq_sl, bk_sl, bvk_sl)):
            for pt in range(2):
                biasT[:, 2 * ti + pt] = bvec[128 * pt:128 * pt + 128]
        biasN = np.stack([bvk_sl, bvv_sl])[None, :, :].astype(np.float32)  # (1,2,DL)

        if BF16:
            import ml_dtypes
            bfc = lambda a: a.astype(ml_dtypes.bfloat16)
            xT, wqT, wkT, wvkT, wvvT = map(bfc, (xT, wqT, wkT, wvkT, wvvT))
        in_maps.append({
            "xT": xT, "wqT": wqT, "wkT": wkT, "wvkT": wvkT, "wvvT": wvvT,
            "woT": woT, "qd": qd_pack, "biasT": biasT, "biasN": biasN,
        })

    global _last_in_maps
    _last_in_maps = in_maps
    res = run_bass_kernel_spmd(nc, in_maps, list(range(8)))
    g0 = res.results[0]["partial"] + res.results[1]["partial"] \
        + res.results[2]["partial"] + res.results[3]["partial"]
    g1 = res.results[4]["partial"] + res.results[5]["partial"] \
        + res.results[6]["partial"] + res.results[7]["partial"]
    out = np.stack([g0 + bo, g1 + bo], axis=1).astype(np.float32)   # (S, B, D)
    return out



# revision 15
# speedup vs baseline: 1.3207x; 1.3207x over previous
"""CompressionAttention TRN2 Bass kernel (8 NeuronCores, SPMD), v2.

Sharding: core c handles batch b = c//4 and heads [4*(c%4), 4*(c%4)+4).
Each core computes its 4 heads' attention output and a partial output
projection (S, D) in bf16; the host sums the 4 partials per batch (fp32)
and adds bo.

Math (see reference): per (b,h)
  w[c,t] = exp(qd_c . k_t)            (max subtraction provably cancels)
  norm[c,s] = cumsum_t w[c,t]
  up[s,c] = (1/norm[c,s]) sum_{t<=s} w[c,t] (q_s . vk_t)
  r~ = softmax_c(up) / norm
  out[s,:] = sum_{t<=s} (sum_c w[c,t] r~[s,c]) vv_t
with vk = x (Wk Wv)^T + bias-fold, vv = x (Wv Wv)^T + bias-fold.

Layouts (per core, 4 heads = 2 pairs j; head p = 2j+m):
  d-indexed tensors pair-packed on 128 partitions: rows 64m..64m+64 = head
  2j+m's 64 dims, one free-slab per pair j (qT2/kT2/vkT2 [128,2,S],
  wo2 [128,2,D], out^T [128,2,128]).
  c-indexed tensors pair-slab on 64 partitions: rows 32m..32m+32 = head
  2j+m's 32 compressed rows (w2/w2m/norm2 [64,2,S], rt [64,2,128],
  kv2 state [64-rows of a 128-part tile]).
  (s, hc) tensors: free col = 64j+32m+c (wT, rn, u, r).
Partition bases stay in {0,32,64} (hardware tile-position constraint).

Tricks that make this fast (cost model: matmul time = moving-dim rows
x cycles/row; bf16 1 cy, fp32r 4 cy if moving<256; elementwise time =
free-size only):
  - vk computed once (natural layout, fused with vv in one [128,512]
    accumulation); vkT2 derived by 16 PE transposes instead of a second
    full projection.
  - w for both heads of a pair in ONE matmul via block-diagonal
    zero-padded qd2 (f32r lhsT, 128-partition contraction).
  - wT = PE-transpose of w2m (no second exp / matmul chain).
  - out-projection contracts 128 partitions per matmul (pair-packed
    lhsT/rhs): half the matmuls of per-head form.
  - all small matmuls take bf16 operands (1 cy/row).
  - exp/cumsum/masks run over all heads at once; staging copies are
    spread across DVE/Act/Pool engines.
"""
import math
import os
import sys

import numpy as np

for _p in ("/opt/trn_rl_repo",):
    if _p not in sys.path and os.path.isdir(_p):
        sys.path.append(_p)

import concourse.bass as bass
import concourse.tile as tile
from concourse import bacc, mybir
from concourse.bass_utils import run_bass_kernel_spmd
from concourse.masks import make_identity, make_upper_triangular

F32 = mybir.dt.float32
F32R = mybir.dt.float32r
BF = mybir.dt.bfloat16
EXP = mybir.ActivationFunctionType.Exp

S, B, D = 1024, 2, 1024
H, DH, C = 16, 64, 32
HPC = 4            # heads per core
DL = HPC * DH      # local head dims = 256
BS = 128           # s-block size
NB = S // BS       # 8 blocks
KT = D // 128      # 8 k-tiles over D
SCALE = 1.0 / math.sqrt(DH)


def _ecopy(eng, out, in_):
    if hasattr(eng, "tensor_copy"):
        eng.tensor_copy(out, in_)
    else:
        eng.copy(out, in_)


def _build_nc(with_bias: bool):
    nc = bacc.Bacc()

    xt_d = nc.declare_dram_parameter("xT", [D, S], BF, isOutput=False)
    wqk_d = nc.declare_dram_parameter("wqk", [D, 512], BF, isOutput=False)
    wvkvv_d = nc.declare_dram_parameter("wvkvv", [D, 512], BF, isOutput=False)
    wo_d = nc.declare_dram_parameter("wo2", [128, 2, D], BF, isOutput=False)
    qd_d = nc.declare_dram_parameter("qd2", [128, 2, 2 * C], F32R, isOutput=False)
    if with_bias:
        biasQK_d = nc.declare_dram_parameter("biasQK", [128, 4], F32, isOutput=False)
        biasN_d = nc.declare_dram_parameter("biasN", [1, 512], F32R, isOutput=False)
    out_d = nc.declare_dram_parameter("partial", [S, D], BF, isOutput=True)

    with tile.TileContext(nc) as tc:
        with tc.tile_pool(name="persist", bufs=1) as pp, \
             tc.tile_pool(name="sloop", bufs=2) as sl:
            # ---- constants ----
            identb = pp.tile([128, 128], BF)
            make_identity(nc, identb)
            identf = pp.tile([64, 64], F32)
            make_identity(nc, identf)
            triu = pp.tile([128, 128], F32)
            make_upper_triangular(nc, triu, val=1.0, diag=True)
            triu4 = pp.tile([128, 4, 128], F32)
            for p in range(HPC):
                nc.gpsimd.tensor_copy(triu4[:, p, :], triu)
            if with_bias:
                ones_f = pp.tile([1, 128], F32)
                nc.vector.memset(ones_f, 1.0)
                ones_col = pp.tile([1, 128], F32R)
                nc.vector.tensor_copy(ones_col, ones_f)

            # ---- persistent SBUF ----
            qT2 = pp.tile([128, 2, S], BF)
            kT2 = pp.tile([128, 2, S], F32R)
            vkT2 = pp.tile([128, 2, S], BF)
            vn = pp.tile([128, NB, 512], BF)     # cols 0:256 vk, 256:512 vv
            w2 = pp.tile([64, 2, S], F32)
            w2m = pp.tile([64, 2, S], BF)
            norm2 = pp.tile([64, 2, S], F32)
            wT = pp.tile([128, NB, 128], BF)     # (t, 64j+32m+c)
            rn = pp.tile([128, NB, 128], F32)    # (s, 64j+32m+c) 1/norm
            wo2 = pp.tile([128, 2, D], BF)

            # ================= Phase 1: loads + projections =================
            with tc.tile_pool(name="p1", bufs=1) as p1, \
                 tc.tile_pool(name="wpool", bufs=2) as wp:
                xt = p1.tile([128, KT, S], BF)
                xt_src = xt_d.rearrange("(kt p) s -> kt p s", p=128)
                for kt in range(KT):
                    eng = (nc.sync, nc.scalar, nc.gpsimd)[kt % 3]
                    eng.dma_start(out=xt[:, kt, :], in_=xt_src[kt])
                qd2 = p1.tile([128, 2, 2 * C], F32R)
                nc.sync.dma_start(out=qd2, in_=qd_d[:, :, :])
                nc.sync.dma_start(out=wo2[:, 0:2, :], in_=wo_d[:, 0:2, :])
                if with_bias:
                    biasQK = p1.tile([128, 4], F32)
                    nc.sync.dma_start(out=biasQK, in_=biasQK_d[:, :])
                    biasN = p1.tile([1, 512], F32R)
                    nc.sync.dma_start(out=biasN, in_=biasN_d[:, :])

                wqk_sb = wp.tile([128, KT, 512], BF, tag="wtile")
                wqk_src = wqk_d.rearrange("(kt p) n -> kt p n", p=128)
                for half in range(2):
                    nc.scalar.dma_start(
                        out=wqk_sb[:, 4 * half:4 * half + 4, :],
                        in_=wqk_src[4 * half:4 * half + 4].rearrange(
                            "kt p n -> p kt n"))
                wvv_sb = wp.tile([128, KT, 512], BF, tag="wtile")
                wvv_src = wvkvv_d.rearrange("(kt p) n -> kt p n", p=128)
                for half in range(2):
                    nc.scalar.dma_start(
                        out=wvv_sb[:, 4 * half:4 * half + 4, :],
                        in_=wvv_src[4 * half:4 * half + 4].rearrange(
                            "kt p n -> p kt n"))

                with tc.tile_pool(name="ps1", bufs=2, space="PSUM") as ps1:
                    # --- k/q projections; k first (feeds w pipeline) ---
                    stage_engs_k = (nc.vector, nc.scalar, nc.vector, nc.scalar)
                    stage_engs_q = (nc.gpsimd, nc.vector, nc.gpsimd, nc.scalar)

                    def _proj(pi, c0, dst):
                        for pt in range(2):
                            for ch in range(2):
                                ps = ps1.tile([128, 512], F32, tag="pproj")
                                for kt in range(KT):
                                    nc.tensor.matmul(
                                        ps,
                                        lhsT=wqk_sb[:, kt, c0 + 128 * pt:
                                                    c0 + 128 * pt + 128],
                                        rhs=xt[:, kt, 512 * ch:512 * ch + 512],
                                        start=(kt == 0), stop=(kt == KT - 1))
                                eng = (stage_engs_k if pi == 0
                                       else stage_engs_q)[2 * pt + ch]
                                if with_bias:
                                    # bias col order: (k0, k1, q0, q1)
                                    nc.vector.tensor_scalar_add(
                                        out=dst[:, pt, 512 * ch:512 * ch + 512],
                                        in0=ps,
                                        scalar1=biasQK[:, 2 * pi + pt:
                                                       2 * pi + pt + 1])
                                else:
                                    _ecopy(eng,
                                           dst[:, pt, 512 * ch:512 * ch + 512],
                                           ps)

                    _proj(0, 256, kT2)
                    # dps: w for both heads of each pair in one matmul
                    with tc.tile_pool(name="psd", bufs=1, space="PSUM") as psd:
                        for ch2 in range(2):
                            ssl = slice(512 * ch2, 512 * ch2 + 512)
                            dps = psd.tile([64, 2, 512], F32, tag="dps")
                            for j in range(2):
                                nc.tensor.matmul(
                                    dps[:, j, :],
                                    lhsT=qd2[:, j, :],
                                    rhs=kT2[:, j, ssl],
                                    start=True, stop=True)
                            nc.scalar.activation(w2[:, :, ssl], dps, func=EXP)
                            nc.scalar.activation(w2m[:, :, ssl], dps, func=EXP)
                        for j in range(2):
                            nc.vector.tensor_tensor_scan(
                                out=norm2[:, j, :], data0=w2[:, j, :],
                                data1=w2[:, j, :], initial=0.0,
                                op0=mybir.AluOpType.add,
                                op1=mybir.AluOpType.bypass)
                    _proj(1, 0, qT2)

                    # --- vk+vv natural projection, one fused group per tt ---
                    with tc.tile_pool(name="pst", bufs=2, space="PSUM") as pst:
                        vkt_prev = None
                        for tt in range(NB):
                            psn = ps1.tile([128, 512], F32, tag="pproj")
                            for kt in range(KT):
                                nc.tensor.matmul(
                                    psn,
                                    lhsT=xt[:, kt, 128 * tt:128 * tt + 128],
                                    rhs=wvv_sb[:, kt, :],
                                    start=(kt == 0),
                                    stop=(kt == KT - 1) and not with_bias)
                            if with_bias:
                                nc.tensor.matmul(psn, lhsT=ones_col,
                                                 rhs=biasN[0:1, :],
                                                 start=False, stop=True)
                            eng = (nc.vector, nc.scalar, nc.gpsimd)[tt % 3]
                            _ecopy(eng, vn[:, tt, :], psn)
                            # vkT2 transposes for the PREVIOUS tt
                            if vkt_prev is not None:
                                _emit_vkt(nc, pst, vn, vkT2, identb, vkt_prev)
                            vkt_prev = tt
                        _emit_vkt(nc, pst, vn, vkT2, identb, vkt_prev)

                        # --- wT + rn per block (needs w2m / norm2) ---
                        for i in range(NB):
                            wt_ps = pst.tile([128, 2, 64], BF, tag="wtT")
                            for j in range(2):
                                nc.tensor.transpose(
                                    wt_ps[:, j, :],
                                    w2m[:, j, 128 * i:128 * i + 128],
                                    identb[0:64, 0:64])
                            nc.vector.tensor_copy(wT[:, i, :], wt_ps)
                            ntp = pst.tile([128, 2, 64], F32, tag="ntp")
                            for j in range(2):
                                nc.tensor.transpose(
                                    ntp[:, j, :],
                                    norm2[:, j, 128 * i:128 * i + 128], identf)
                            nc.vector.reciprocal_approx_fast(
                                out=rn[:, i, :], in_=ntp)

            # ================= Phase 3: blocked attention =================
            with tc.tile_pool(name="psg", bufs=1, space="PSUM") as psg, \
                 tc.tile_pool(name="psu", bufs=1, space="PSUM") as psu, \
                 tc.tile_pool(name="psm", bufs=1, space="PSUM") as psm, \
                 tc.tile_pool(name="psx", bufs=1, space="PSUM") as psx, \
                 tc.tile_pool(name="pso", bufs=1, space="PSUM") as pso, \
                 tc.tile_pool(name="psp", bufs=2, space="PSUM") as psp, \
                 tc.tile_pool(name="pop", bufs=2) as pop:
                kvs_f = None
                kvs_r = None
                for i in range(NB):
                    blk = slice(128 * i, 128 * i + 128)
                    # gt: within-block vk.q per head
                    gt_ps = psg.tile([128, 4, 128], F32, tag="gt")
                    for p in range(HPC):
                        j, m = p // 2, p % 2
                        dsl = slice(64 * m, 64 * m + 64)
                        nc.tensor.matmul(
                            gt_ps[:, p, :],
                            lhsT=vkT2[dsl, j, blk],
                            rhs=qT2[dsl, j, blk],
                            start=True, stop=True)
                    gt_sb = sl.tile([128, 4, 128], BF, tag="gtsb")
                    (nc.gpsimd if i % 2 else nc.vector).tensor_mul(
                        gt_sb, gt_ps, triu4)
                    # u = prefix (q.KVW) + diag
                    u_ps = psu.tile([128, 128], F32, tag="u")
                    for p in range(HPC):
                        j, m = p // 2, p % 2
                        dsl = slice(64 * m, 64 * m + 64)
                        csl = slice(32 * p, 32 * p + 32)
                        if i > 0:
                            nc.tensor.matmul(
                                u_ps[:, csl],
                                lhsT=qT2[dsl, j, blk],
                                rhs=kvs_r[dsl, 32 * j:32 * j + 32],
                                start=True, stop=False)
                        nc.tensor.matmul(
                            u_ps[:, csl],
                            lhsT=gt_sb[:, p, :],
                            rhs=wT[:, i, csl],
                            start=(i == 0), stop=True)
                    # softmax over c (32/head), folded with 1/norm
                    up_sb = sl.tile([128, 128], F32, tag="up")
                    nc.vector.tensor_mul(up_sb, u_ps, rn[:, i, :])
                    e_sb = sl.tile([128, 128], F32, tag="e")
                    nc.scalar.activation(e_sb, up_sb, func=EXP)
                    sum_sb = sl.tile([128, HPC], F32, tag="sum")
                    nc.vector.tensor_reduce(
                        sum_sb, e_sb[:, :].rearrange("a (h c) -> a h c", h=HPC),
                        axis=mybir.AxisListType.X, op=mybir.AluOpType.add)
                    inv_sb = sl.tile([128, HPC], F32, tag="inv")
                    nc.vector.reciprocal(inv_sb, sum_sb)
                    r_sb = sl.tile([128, 128], BF, tag="r")
                    for p in range(HPC):
                        csl = slice(32 * p, 32 * p + 32)
                        nc.vector.scalar_tensor_tensor(
                            out=r_sb[:, csl],
                            in0=e_sb[:, csl],
                            scalar=inv_sb[:, p:p + 1],
                            in1=rn[:, i, csl],
                            op0=mybir.AluOpType.mult,
                            op1=mybir.AluOpType.mult)
                    # r~^T per pair (rows 32m+c)
                    rt_ps = psx.tile([64, 2, 128], BF, tag="rt")
                    for j in range(2):
                        nc.tensor.transpose(
                            rt_ps[:, j, :], r_sb[:, 64 * j:64 * j + 64], identb)
                    rt_sb = sl.tile([64, 2, 128], BF, tag="rtsb")
                    nc.vector.tensor_copy(rt_sb, rt_ps)
                    # mt[t,s] = sum_c w[c,t] r~[s,c] per head, masked
                    mt_ps = psm.tile([128, 4, 128], F32, tag="mt")
                    for p in range(HPC):
                        j, m = p // 2, p % 2
                        msl = slice(32 * m, 32 * m + 32)
                        nc.tensor.matmul(
                            mt_ps[:, p, :],
                            lhsT=w2m[msl, j, blk],
                            rhs=rt_sb[msl, j, :],
                            start=True, stop=True)
                    mt_sb = sl.tile([128, 4, 128], BF, tag="mtsb")
                    (nc.vector if i % 2 else nc.gpsimd).tensor_mul(
                        mt_sb, mt_ps, triu4)
                    # state deltas: KVW (cols 0:64), KV2 pair slabs (64:192)
                    dl_ps = psx.tile([128, 192], F32, tag="delta")
                    for p in range(HPC):
                        j, m = p // 2, p % 2
                        dsl = slice(64 * m, 64 * m + 64)
                        msl = slice(32 * m, 32 * m + 32)
                        csl = slice(32 * p, 32 * p + 32)
                        nc.tensor.matmul(
                            dl_ps[dsl, 32 * j:32 * j + 32],
                            lhsT=vn[:, i, 64 * p:64 * p + 64],
                            rhs=wT[:, i, csl],
                            start=True, stop=True)
                        nc.tensor.matmul(
                            dl_ps[msl, 64 + 64 * j:64 + 64 * j + 64],
                            lhsT=wT[:, i, csl],
                            rhs=vn[:, i, 256 + 64 * p:256 + 64 * p + 64],
                            start=True, stop=True)
                    # out^T = prefix (KV2.r~) + diag (vv.mt), pair-packed
                    ot_ps = pso.tile([128, 2, 128], F32, tag="ot")
                    for p in range(HPC):
                        j, m = p // 2, p % 2
                        dsl = slice(64 * m, 64 * m + 64)
                        msl = slice(32 * m, 32 * m + 32)
                        if i > 0:
                            nc.tensor.matmul(
                                ot_ps[dsl, j, :],
                                lhsT=kvs_r[msl, 64 + 64 * j:64 + 64 * j + 64],
                                rhs=rt_sb[msl, j, :],
                                start=True, stop=False)
                        nc.tensor.matmul(
                            ot_ps[dsl, j, :],
                            lhsT=vn[:, i, 256 + 64 * p:256 + 64 * p + 64],
                            rhs=mt_sb[:, p, :],
                            start=(i == 0), stop=True)
                    ot_sb = sl.tile([128, 2, 128], BF, tag="otsb")
                    nc.scalar.copy(ot_sb, ot_ps)
                    # interleaved output projection
                    for ch in range(2):
                        po = psp.tile([128, 512], F32, tag="po")
                        for j in range(2):
                            nc.tensor.matmul(
                                po,
                                lhsT=ot_sb[:, j, :],
                                rhs=wo2[:, j, 512 * ch:512 * ch + 512],
                                start=(j == 0), stop=(j == 1))
                        po_sb = pop.tile([128, 512], BF, tag="posb")
                        if (2 * i + ch) % 2 == 0:
                            nc.scalar.copy(po_sb, po)
                        else:
                            nc.gpsimd.tensor_copy(po_sb, po)
                        nc.sync.dma_start(
                            out=out_d[128 * i:128 * i + 128,
                                      512 * ch:512 * ch + 512],
                            in_=po_sb)
                    # advance running state
                    if i < NB - 1:
                        kvs_new = sl.tile([128, 192], F32, tag="kvsf")
                        if i == 0:
                            nc.vector.tensor_copy(
                                kvs_new[:, 0:64], dl_ps[:, 0:64])
                            nc.vector.tensor_copy(
                                kvs_new[0:64, 64:192], dl_ps[0:64, 64:192])
                        else:
                            nc.vector.tensor_add(
                                kvs_new[:, 0:64], kvs_f[:, 0:64],
                                dl_ps[:, 0:64])
                            nc.vector.tensor_add(
                                kvs_new[0:64, 64:192], kvs_f[0:64, 64:192],
                                dl_ps[0:64, 64:192])
                        kvs_f = kvs_new
                        kvs_r = sl.tile([128, 192], BF, tag="kvsr")
                        nc.scalar.copy(kvs_r[:, 0:64], kvs_f[:, 0:64])
                        nc.scalar.copy(kvs_r[0:64, 64:192], kvs_f[0:64, 64:192])

    nc.finalize()
    return nc


def _emit_vkt(nc, pst, vn, vkT2, identb, tt):
    vk_ps = pst.tile([128, 2, 128], BF, tag="vkT")
    for j in range(2):
        nc.tensor.transpose(
            vk_ps[:, j, :], vn[:, tt, 128 * j:128 * j + 128], identb)
    nc.vector.tensor_copy(vkT2[:, :, 128 * tt:128 * tt + 128], vk_ps)


_NC_CACHE = {}


def _get_nc(with_bias: bool = False):
    if with_bias not in _NC_CACHE:
        _NC_CACHE[with_bias] = _build_nc(with_bias)
    return _NC_CACHE[with_bias]


def kernel(x, q_down, Wq, bq, Wk, bk, Wv, bv, Wo, bo):
    import ml_dtypes
    bfc = lambda a: np.ascontiguousarray(a).astype(ml_dtypes.bfloat16)

    x = np.asarray(x, np.float32)
    q_down = np.asarray(q_down, np.float32)
    Wq = np.asarray(Wq, np.float32); bq = np.asarray(bq, np.float32)
    Wk = np.asarray(Wk, np.float32); bk = np.asarray(bk, np.float32)
    Wv = np.asarray(Wv, np.float32); bv = np.asarray(bv, np.float32)
    Wo = np.asarray(Wo, np.float32); bo = np.asarray(bo, np.float32)

    with_bias = bool(np.any(bq) or np.any(bk) or np.any(bv))
    nc = _get_nc(with_bias)
    qd_r = q_down.reshape(C, H, DH)

    in_maps = []
    for core in range(8):
        b = core // 4
        h0 = HPC * (core % 4)
        hsl = slice(h0 * DH, (h0 + HPC) * DH)

        xT = bfc(x[:, b, :].T)                                  # (D, S)
        wqT = (SCALE * Wq[hsl, :]).T                            # (D, 256)
        wkT = Wk[hsl, :].T
        wqk = bfc(np.concatenate([wqT, wkT], axis=1))           # (D, 512)
        wvkT = (Wk[hsl, :] @ Wv).T
        wvvT = (Wv[hsl, :] @ Wv).T
        wvkvv = bfc(np.concatenate([wvkT, wvvT], axis=1))       # (D, 512)
        # wo2 pair-packed: [128, 2, D]
        wo2 = bfc(Wo[:, hsl].T.reshape(2, 128, D).transpose(1, 0, 2))
        # qd2 block-diag pair lhsT: [128, 2, 64]
        qd2 = np.zeros((128, 2, 2 * C), np.float32)
        for j in range(2):
            qd2[0:64, j, 0:C] = SCALE * qd_r[:, h0 + 2 * j, :].T
            qd2[64:128, j, C:2 * C] = SCALE * qd_r[:, h0 + 2 * j + 1, :].T

        im = {"xT": xT, "wqk": wqk, "wvkvv": wvkvv, "wo2": wo2, "qd2": qd2}
        if with_bias:
            bq_sl = SCALE * bq[hsl]
            bk_sl = bk[hsl]
            bvk_sl = Wk[hsl, :] @ bv + bk_sl
            bvv_sl = Wv[hsl, :] @ bv + bv[hsl]
            biasQK = np.zeros((128, 4), np.float32)
            for pt in range(2):
                biasQK[:, 0 + pt] = bq_sl[128 * pt:128 * pt + 128]
                biasQK[:, 2 + pt] = bk_sl[128 * pt:128 * pt + 128]
            im["biasQK"] = biasQK
            im["biasN"] = np.concatenate([bvk_sl, bvv_sl])[None, :].astype(
                np.float32)
        in_maps.append(im)

    res = run_bass_kernel_spmd(nc, in_maps, list(range(8)))
    parts = [res.results[c]["partial"].astype(np.float32) for c in range(8)]
    g0 = parts[0] + parts[1] + parts[2] + parts[3]
    g1 = parts[4] + parts[5] + parts[6] + parts[7]
    out = np.stack([g0 + bo, g1 + bo], axis=1).astype(np.float32)   # (S, B, D)
    return out
